# revision 1
# baseline (speedup 1.0000x reference)
"""Trainium2 Bass kernel for the SSD-style detection loss (nn_DetectionLoss).

Self-contained: hardcodes shapes/sharding for the full problem
(B=16, C=3, A=9, H=128, W=128, M=32) and shards the batch across 8 cores.

kernel(**inputs) takes the FULL inputs and returns the FULL [4] f32 output.

Per-core pipeline (per image):
  1. pair stage: packed best-IoU over the M gt boxes
     key = [iou 25 bits | (31-m) 5 bits | (label-1) 2 bits] as a valid f32;
     a single f32 max-reduce over m yields best-iou, argmax and matched class.
  2. pos/neg masks, objectness BCE terms (softplus via relu + ln(1+exp(-|x|))).
  3. hard-negative mining: branch-free 30-step binary search for the k-th
     largest neg logit, then one fused masked count+sum with a tie correction.
  4. matched-box payloads via a 32-step select loop, then smooth-L1 loc loss
     and CE cls loss as fused masked reductions.
Partial sums [obj, cls, loc, num_pos, num_sel] go back to the host, which
adds the 8 cores and applies the final normalization.
"""
import numpy as np

import concourse.bass as bass
import concourse.bacc as bacc
import concourse.mybir as mybir
from concourse.tile import TileContext
from concourse.bass_utils import run_bass_kernel_spmd

F32 = mybir.dt.float32
U32 = mybir.dt.uint32
I32 = mybir.dt.int32
OP = mybir.AluOpType
AF = mybir.ActivationFunctionType
AX = mybir.AxisListType

# Full-problem constants
B, C, A, H, W, M = 16, 3, 9, 128, 128, 32
K = 5 + C
N_CORES = 8
N_IMG = B // N_CORES

# Packing layout in the f32 iou key: [iou 25 bits | (31-m) 5 bits | (label-1) 2 bits]
PK_MASK = 0xFFFFFF80
IDX_MASK = 0x7C
TGT_MASK = 0x3
NEG_FILL = -1.0e30
N_SEARCH = 30
SEARCH_SPAN = 8.0  # binary search reaches [-2*SPAN, 2*SPAN]
import os as _os
STOP_STAGE = int(_os.environ.get("KK_STOP_STAGE", "99"))


def _stt_u32imm(eng, out, in0, imm, in1, op0, op1):
    """scalar_tensor_tensor with an integer-typed immediate (bitvec ops)."""
    return eng.add_instruction(
        mybir.InstTensorScalarPtr(
            name=eng.bass.get_next_instruction_name(),
            is_scalar_tensor_tensor=True,
            op0=op0, op1=op1,
            ins=[eng.lower_ap(in0),
                 mybir.ImmediateValue(dtype=U32, value=imm),
                 eng.lower_ap(in1)],
            outs=[eng.lower_ap(out)],
        ))


def _ts_u32imm(eng, out, in_, imm, op):
    """tensor_scalar single-op with an integer-typed immediate (bitvec ops)."""
    return eng.add_instruction(
        mybir.InstTensorScalarPtr(
            name=eng.bass.get_next_instruction_name(),
            op0=op, op1=OP.bypass,
            ins=[eng.lower_ap(in_),
                 mybir.ImmediateValue(dtype=U32, value=imm)],
            outs=[eng.lower_ap(out)],
        ))


def build_nc(n_img=N_IMG, Wp=W, Ap=A, Mp=M, Cp=C, n_search=N_SEARCH, wc=32):
    """Build the per-core SPMD program. Partition dim is H=128 (fixed)."""
    Hp = H
    Kp = 5 + Cp
    AW = Ap * Wp
    assert Wp % wc == 0
    nw = Wp // wc

    nc = bacc.Bacc("TRN2", target_bir_lowering=False, debug=False)
    d_pred = nc.declare_dram_parameter("pred", [n_img, Ap * Kp, Hp, Wp], F32, isOutput=False)
    d_anc = nc.declare_dram_parameter("anc", [4, Hp, AW], F32, isOutput=False)
    d_gtb = nc.declare_dram_parameter("gtb", [n_img, 4, Hp, Mp], F32, isOutput=False)
    d_gtl = nc.declare_dram_parameter("gtl", [n_img, Hp, Mp], F32, isOutput=False)
    d_ipat = nc.declare_dram_parameter("ipat", [Hp, Mp], U32, isOutput=False)
    d_res = nc.declare_dram_parameter("res", [1, 8], F32, isOutput=True)

    V, G, S = nc.vector, nc.gpsimd, nc.scalar

    with TileContext(nc) as tc:
        with (
            tc.tile_pool(name="anc_pool", bufs=1) as pa,     # persistent anchor planes
            tc.tile_pool(name="pred_pool", bufs=1) as pp,    # pred image tile
            tc.tile_pool(name="pair_pool", bufs=5) as pr,    # pair-stage scratch (wc*Mp)
            tc.tile_pool(name="scr_pool", bufs=4) as ps,     # [Hp, AW] scratch
            tc.tile_pool(name="plane_pool", bufs=1) as pl,   # [Hp, AW] per-image planes
            tc.tile_pool(name="gt_pool", bufs=2) as pg,      # [Hp, Mp] gt tables
            tc.tile_pool(name="tiny_pool", bufs=2) as pt,    # [128,1] / [1,1] scalars
        ):
            def scr(tag="tmpA"):
                bufs = 1 if tag == "junkN" else None
                return ps.tile([Hp, AW], F32, tag=tag, name=tag, bufs=bufs)

            # ---------- anchor-derived planes (shared across images) ----------
            ax1 = pa.tile([Hp, AW], F32)
            ay1 = pa.tile([Hp, AW], F32)
            ax2 = pa.tile([Hp, AW], F32)
            ay2 = pa.tile([Hp, AW], F32)
            nc.sync.dma_start(out=ax1[:, :], in_=d_anc[0])
            nc.sync.dma_start(out=ay1[:, :], in_=d_anc[1])
            nc.sync.dma_start(out=ax2[:, :], in_=d_anc[2])
            nc.sync.dma_start(out=ay2[:, :], in_=d_anc[3])

            awr = scr()
            V.tensor_sub(out=awr[:, :], in0=ax2[:, :], in1=ax1[:, :])
            ahr = scr()
            V.tensor_sub(out=ahr[:, :], in0=ay2[:, :], in1=ay1[:, :])
            areaA = pa.tile([Hp, AW], F32)
            V.tensor_mul(out=areaA[:, :], in0=awr[:, :], in1=ahr[:, :])
            awc = scr()
            V.tensor_scalar_max(out=awc[:, :], in0=awr[:, :], scalar1=1e-6)
            ahc = scr()
            V.tensor_scalar_max(out=ahc[:, :], in0=ahr[:, :], scalar1=1e-6)
            invaw = pa.tile([Hp, AW], F32)
            V.reciprocal(out=invaw[:, :], in_=awc[:, :])
            invah = pa.tile([Hp, AW], F32)
            V.reciprocal(out=invah[:, :], in_=ahc[:, :])
            logaw = pa.tile([Hp, AW], F32)
            S.activation(out=logaw[:, :], in_=awc[:, :], func=AF.Ln)
            logah = pa.tile([Hp, AW], F32)
            S.activation(out=logah[:, :], in_=ahc[:, :], func=AF.Ln)
            # axw = acx * invaw ; ayh = acy * invah
            t0 = scr()
            V.tensor_add(out=t0[:, :], in0=ax1[:, :], in1=ax2[:, :])
            t0h = scr()
            V.tensor_scalar_mul(out=t0h[:, :], in0=t0[:, :], scalar1=0.5)
            axw = pa.tile([Hp, AW], F32)
            V.tensor_mul(out=axw[:, :], in0=t0h[:, :], in1=invaw[:, :])
            t1a = scr()
            V.tensor_add(out=t1a[:, :], in0=ay1[:, :], in1=ay2[:, :])
            t1h = scr()
            V.tensor_scalar_mul(out=t1h[:, :], in0=t1a[:, :], scalar1=0.5)
            ayh = pa.tile([Hp, AW], F32)
            V.tensor_mul(out=ayh[:, :], in0=t1h[:, :], in1=invah[:, :])

            ipatb = pa.tile([Hp, Mp], U32)
            nc.sync.dma_start(out=ipatb[:, :], in_=d_ipat[:, :])

            per_image_scalars = []  # (objpos, objneg, cls, loc, npos, k)

            for i in range(n_img):
                # ---------- per-image gt tables [Hp, Mp] ----------
                def gtile(tag):
                    return pg.tile([Hp, Mp], F32, tag=tag, name=tag)

                gx1, gy1, gx2, gy2 = gtile("gx1"), gtile("gy1"), gtile("gx2"), gtile("gy2")
                nc.sync.dma_start(out=gx1[:, :], in_=d_gtb[i, 0])
                nc.sync.dma_start(out=gy1[:, :], in_=d_gtb[i, 1])
                nc.sync.dma_start(out=gx2[:, :], in_=d_gtb[i, 2])
                nc.sync.dma_start(out=gy2[:, :], in_=d_gtb[i, 3])
                lab = gtile("lab")
                nc.sync.dma_start(out=lab[:, :], in_=d_gtl[i])

                gwr, ghr = gtile("gwr"), gtile("ghr")
                V.tensor_sub(out=gwr[:, :], in0=gx2[:, :], in1=gx1[:, :])
                V.tensor_sub(out=ghr[:, :], in0=gy2[:, :], in1=gy1[:, :])
                areaB = gtile("areaB")
                V.tensor_mul(out=areaB[:, :], in0=gwr[:, :], in1=ghr[:, :])
                tg1, tg2 = gtile("tg1"), gtile("tg2")
                V.tensor_add(out=tg1[:, :], in0=gx1[:, :], in1=gx2[:, :])
                V.tensor_add(out=tg2[:, :], in0=gy1[:, :], in1=gy2[:, :])
                gcx, gcy = gtile("gcx"), gtile("gcy")
                V.tensor_scalar_mul(out=gcx[:, :], in0=tg1[:, :], scalar1=0.5)
                V.tensor_scalar_mul(out=gcy[:, :], in0=tg2[:, :], scalar1=0.5)
                gwc, ghc = gtile("gwc"), gtile("ghc")
                V.tensor_scalar_max(out=gwc[:, :], in0=gwr[:, :], scalar1=1e-6)
                V.tensor_scalar_max(out=ghc[:, :], in0=ghr[:, :], scalar1=1e-6)
                lgw, lgh = gtile("lgw"), gtile("lgh")
                S.activation(out=lgw[:, :], in_=gwc[:, :], func=AF.Ln)
                S.activation(out=lgh[:, :], in_=ghc[:, :], func=AF.Ln)
                # packed idx|tgt pattern: ipat | clip(label-1, 0, C-1)
                lm1 = gtile("lm1")
                V.tensor_scalar(out=lm1[:, :], in0=lab[:, :], scalar1=1.0,
                                scalar2=0.0, op0=OP.subtract, op1=OP.max)
                lm1c = gtile("lm1c")
                V.tensor_scalar_min(out=lm1c[:, :], in0=lm1[:, :], scalar1=float(Cp - 1))
                lm1u = pg.tile([Hp, Mp], U32, tag="lm1u", name="lm1u")
                V.tensor_copy(out=lm1u[:, :], in_=lm1c[:, :])
                ipat = pg.tile([Hp, Mp], U32, tag="ipat", name="ipat")
                V.tensor_tensor(out=ipat[:, :], in0=ipatb[:, :], in1=lm1u[:, :],
                                op=OP.bitwise_or)

                # ---------- pair stage: packed best-iou ----------
                pmax = pl.tile([Hp, AW], F32, tag="pmax", name="pmax")
                for a in range(Ap):
                    for wci in range(nw):
                        lo = a * Wp + wci * wc
                        sl = slice(lo, lo + wc)

                        def ab(t):  # anchor plane slice -> [Hp, wc, Mp] bcast
                            return t[:, sl].unsqueeze(2).broadcast_to([Hp, wc, Mp])

                        def gb(t):  # gt table -> [Hp, wc, Mp] bcast
                            return t[:, :].unsqueeze(1).broadcast_to([Hp, wc, Mp])

                        def ptile(nm):
                            t = pr.tile([Hp, wc * Mp], F32, tag="ptmp", name=nm)
                            return t, t.rearrange("p (w m) -> p w m", m=Mp)

                        t1, t13 = ptile("t1")
                        V.tensor_tensor(out=t13, in0=ab(ax2), in1=gb(gx2), op=OP.min)
                        t2, t23 = ptile("t2")
                        V.tensor_tensor(out=t23, in0=ab(ax1), in1=gb(gx1), op=OP.max)
                        wx, wx3 = ptile("wx")
                        V.tensor_tensor(out=wx3, in0=t13, in1=t23, op=OP.subtract)

                        t3, t33 = ptile("t3")
                        V.tensor_tensor(out=t33, in0=ab(ay2), in1=gb(gy2), op=OP.min)
                        t4, t43 = ptile("t4")
                        V.tensor_tensor(out=t43, in0=ab(ay1), in1=gb(gy1), op=OP.max)
                        wy, wy3 = ptile("wy")
                        V.tensor_tensor(out=wy3, in0=t33, in1=t43, op=OP.subtract)
                        wyc, wyc3 = ptile("wyc")
                        S.activation(out=wyc[:, :], in_=wy[:, :], func=AF.Relu)

                        inter, inter3 = ptile("inter")
                        V.scalar_tensor_tensor(out=inter3, in0=wx3, scalar=0.0,
                                               in1=wyc3, op0=OP.max, op1=OP.mult)
                        nu, nu3 = ptile("nu")
                        V.scalar_tensor_tensor(out=nu3, in0=inter3, scalar=-1.0,
                                               in1=ab(areaA), op0=OP.mult, op1=OP.add)
                        union, un3 = ptile("union")
                        V.tensor_tensor(out=un3, in0=nu3, in1=gb(areaB), op=OP.add)
                        unc, unc3 = ptile("unc")
                        V.tensor_scalar_max(out=unc[:, :], in0=union[:, :], scalar1=1e-9)
                        runc, runc3 = ptile("runc")
                        V.reciprocal(out=runc[:, :], in_=unc[:, :])
                        iou, iou3 = ptile("iou")
                        V.tensor_tensor(out=iou3, in0=inter3, in1=runc3, op=OP.mult)
                        pk, pk3 = ptile("pk")
                        _stt_u32imm(
                            V, pk3.bitcast(U32), iou3.bitcast(U32), PK_MASK,
                            ipat[:, :].unsqueeze(1).broadcast_to([Hp, wc, Mp]),
                            OP.bitwise_and, OP.bitwise_or)
                        V.tensor_reduce(out=pmax[:, sl], in_=pk3, axis=AX.X, op=OP.max)

                if STOP_STAGE < 2:
                    per_image_scalars.append(None)
                    continue
                # ---------- post-match masks ----------
                pmu = pmax.bitcast(U32)
                bq = scr()
                _ts_u32imm(V, bq[:, :].bitcast(U32), pmu[:, :], PK_MASK, OP.bitwise_and)
                pos_f = pl.tile([Hp, AW], F32, tag="pos_f", name="pos_f")
                V.tensor_single_scalar(out=pos_f[:, :], in_=bq[:, :], scalar=0.5, op=OP.is_ge)
                neg_f = pl.tile([Hp, AW], F32, tag="neg_f", name="neg_f")
                V.tensor_single_scalar(out=neg_f[:, :], in_=bq[:, :], scalar=0.3, op=OP.is_lt)

                # ---------- pred tile (needed from here on) ----------
                pred_t = pp.tile([Hp, Kp * Ap * Wp], F32, tag="pred", name="pred_t")
                nc.sync.dma_start(out=pred_t[:, :], in_=d_pred[i].transpose([1, 0, 2]))
                pv = pred_t.rearrange("p (a k w) -> p a k w", a=Ap, k=Kp)
                pobj = pv[:, :, 4, :]

                # ---------- objectness terms ----------
                axp = scr()
                S.activation(out=axp[:, :], in_=pobj, func=AF.Abs)
                exn = scr()
                S.activation(out=exn[:, :], in_=axp[:, :], func=AF.Exp, scale=-1.0)
                lgp = scr()
                S.activation(out=lgp[:, :], in_=exn[:, :], func=AF.Ln, bias=1.0)
                rln = scr()
                S.activation(out=rln[:, :], in_=pobj, func=AF.Relu, scale=-1.0)
                rlp = scr()
                S.activation(out=rlp[:, :], in_=pobj, func=AF.Relu)
                spneg = scr()
                V.tensor_add(out=spneg[:, :], in0=rln[:, :], in1=lgp[:, :])
                objpos_col = pt.tile([Hp, 1], F32, tag="objpos_col", name="objpos_col")
                junk = scr("junkN")
                V.tensor_mul(out=junk[:, :], in0=spneg[:, :], in1=pos_f[:, :])
                V.tensor_reduce(out=objpos_col[:, :], in_=junk[:, :], axis=AX.X, op=OP.add)
                objpos_t = pt.tile([1, 1], F32, tag="objpos_t", name="objpos_t")
                G.tensor_reduce(out=objpos_t[:1, :1], in_=objpos_col[:, :], axis=AX.C, op=OP.add)
                # spz = softplus(z) where z = pobj masked to neg, else 0
                lz = scr()
                V.tensor_mul(out=lz[:, :], in0=lgp[:, :], in1=neg_f[:, :])
                rz = scr()
                V.tensor_mul(out=rz[:, :], in0=rlp[:, :], in1=neg_f[:, :])
                spz = pl.tile([Hp, AW], F32, tag="spz", name="spz")
                V.tensor_add(out=spz[:, :], in0=lz[:, :], in1=rz[:, :])
                # zt = neg ? pobj : NEG_FILL   (exact arithmetic blend)
                zt = pl.tile([Hp, AW], F32, tag="zt", name="zt")
                zmask = scr("tmpB")
                V.tensor_mul(out=zmask[:, :], in0=neg_f[:, :], in1=pobj)
                zfill = scr("tmpB")
                V.tensor_scalar(out=zfill[:, :], in0=neg_f[:, :], scalar1=1.0,
                                scalar2=-NEG_FILL, op0=OP.subtract, op1=OP.mult)
                V.tensor_add(out=zt[:, :], in0=zmask[:, :], in1=zfill[:, :])

                if STOP_STAGE < 3:
                    per_image_scalars.append(None)
                    continue
                # ---------- counts and k ----------
                def tiny(tag, dt=F32):
                    return pt.tile([1, 1], dt, tag=tag, name=tag)

                npos_col = pt.tile([Hp, 1], F32, tag="npos_col", name="npos_col")
                V.tensor_reduce(out=npos_col[:, :], in_=pos_f[:, :], axis=AX.X, op=OP.add)
                nneg_col = pt.tile([Hp, 1], F32, tag="nneg_col", name="nneg_col")
                V.tensor_reduce(out=nneg_col[:, :], in_=neg_f[:, :], axis=AX.X, op=OP.add)
                npos_t = tiny("npos_t")
                G.tensor_reduce(out=npos_t[:1, :1], in_=npos_col[:, :], axis=AX.C, op=OP.add)
                nneg_t = tiny("nneg_t")
                G.tensor_reduce(out=nneg_t[:1, :1], in_=nneg_col[:, :], axis=AX.C, op=OP.add)
                # k = npos==0 ? (nneg>0 ? max(floor(nneg/10),1) : 0) : min(3*npos, nneg)
                np3 = tiny("np3")
                V.tensor_scalar_mul(out=np3[:, :], in0=npos_t[:, :], scalar1=3.0)
                kmin = tiny("kmin")
                V.tensor_tensor(out=kmin[:, :], in0=np3[:, :], in1=nneg_t[:, :], op=OP.min)
                nn10 = tiny("nn10")
                V.tensor_scalar_mul(out=nn10[:, :], in0=nneg_t[:, :], scalar1=0.1)
                nn10i = tiny("nn10i", I32)
                V.tensor_copy(out=nn10i[:, :], in_=nn10[:, :])
                nn10f = tiny("nn10f")
                V.tensor_copy(out=nn10f[:, :], in_=nn10i[:, :])
                k2 = tiny("k2")
                V.tensor_scalar_max(out=k2[:, :], in0=nn10f[:, :], scalar1=1.0)
                znn = tiny("znn")
                V.tensor_single_scalar(out=znn[:, :], in_=nneg_t[:, :], scalar=0.0, op=OP.is_gt)
                k2z = tiny("k2z")
                V.tensor_mul(out=k2z[:, :], in0=k2[:, :], in1=znn[:, :])
                zf = tiny("zf")
                V.tensor_single_scalar(out=zf[:, :], in_=npos_t[:, :], scalar=0.0, op=OP.is_equal)
                kd = tiny("kd")
                V.tensor_sub(out=kd[:, :], in0=k2z[:, :], in1=kmin[:, :])
                kzd = tiny("kzd")
                V.tensor_mul(out=kzd[:, :], in0=zf[:, :], in1=kd[:, :])
                kk = tiny("kk")
                V.tensor_add(out=kk[:, :], in0=kmin[:, :], in1=kzd[:, :])

                if STOP_STAGE < 4:
                    per_image_scalars.append(None)
                    continue
                # ---------- binary search for the k-th largest z ----------
                thb = pt.tile([Hp, 1], F32, tag="thb", name="thb")
                V.memset(thb[:, :], 0.0)
                th1 = tiny("th1")
                V.memset(th1[:, :], 0.0)
                for it in range(n_search):
                    s_i = SEARCH_SPAN * (0.5 ** it)
                    cnt_col = pt.tile([Hp, 1], F32, tag="cnt_col", name="cnt_col")
                    junk = scr("junkN")
                    V.tensor_scalar(out=junk[:, :], in0=zt[:, :], scalar1=thb[:, :],
                                    scalar2=None, op0=OP.is_gt, op1=OP.add,
                                    accum_out=cnt_col[:, :])
                    cnt_t = tiny("cnt_t")
                    G.tensor_reduce(out=cnt_t[:1, :1], in_=cnt_col[:, :], axis=AX.C, op=OP.add)
                    ge = tiny("ge")
                    V.tensor_tensor(out=ge[:, :], in0=cnt_t[:, :], in1=kk[:, :], op=OP.is_ge)
                    # th += s * (2*ge - 1)
                    V.scalar_tensor_tensor(out=th1[:, :], in0=ge[:, :], scalar=2.0 * s_i,
                                           in1=th1[:, :], op0=OP.mult, op1=OP.add)
                    V.tensor_scalar_sub(out=th1[:, :], in0=th1[:, :], scalar1=s_i)
                    G.partition_broadcast(thb[:, :], th1[:1, :1], channels=Hp)

                if STOP_STAGE < 5:
                    per_image_scalars.append(None)
                    continue
                # final masked count + sum of softplus over selected negs
                cntF_col = pt.tile([Hp, 1], F32, tag="cntF_col", name="cntF_col")
                junk = scr("junkN")
                V.tensor_scalar(out=junk[:, :], in0=zt[:, :], scalar1=thb[:, :],
                                scalar2=None, op0=OP.is_gt, op1=OP.add,
                                accum_out=cntF_col[:, :])
                selsum_col = pt.tile([Hp, 1], F32, tag="selsum_col", name="selsum_col")
                junk = scr("junkN")
                V.scalar_tensor_tensor(out=junk[:, :], in0=zt[:, :], scalar=thb[:, :],
                                       in1=spz[:, :], op0=OP.is_gt, op1=OP.mult)
                V.tensor_reduce(out=selsum_col[:, :], in_=junk[:, :], axis=AX.X, op=OP.add)
                cntF_t = tiny("cntF_t")
                G.tensor_reduce(out=cntF_t[:1, :1], in_=cntF_col[:, :], axis=AX.C, op=OP.add)
                selsum_t = tiny("selsum_t")
                G.tensor_reduce(out=selsum_t[:1, :1], in_=selsum_col[:, :], axis=AX.C, op=OP.add)
                # softplus(th) for the duplicate/resolution correction
                tha = tiny("tha")
                S.activation(out=tha[:, :], in_=th1[:, :], func=AF.Abs)
                the = tiny("the")
                S.activation(out=the[:, :], in_=tha[:, :], func=AF.Exp, scale=-1.0)
                thl = tiny("thl")
                S.activation(out=thl[:, :], in_=the[:, :], func=AF.Ln, bias=1.0)
                thr = tiny("thr")
                S.activation(out=thr[:, :], in_=th1[:, :], func=AF.Relu)
                sth = tiny("sth")
                V.tensor_add(out=sth[:, :], in0=thr[:, :], in1=thl[:, :])
                kc = tiny("kc")
                V.tensor_sub(out=kc[:, :], in0=kk[:, :], in1=cntF_t[:, :])
                kcs = tiny("kcs")
                V.tensor_mul(out=kcs[:, :], in0=kc[:, :], in1=sth[:, :])
                objneg_t = tiny("objneg_t")
                V.tensor_add(out=objneg_t[:, :], in0=selsum_t[:, :], in1=kcs[:, :])

                # ---------- classification ----------
                pc = [pv[:, :, 5 + c, :] for c in range(Cp)]
                ex0 = scr()
                S.activation(out=ex0[:, :], in_=pc[0], func=AF.Exp)
                ex1 = scr()
                S.activation(out=ex1[:, :], in_=pc[1], func=AF.Exp)
                es01 = scr()
                V.tensor_add(out=es01[:, :], in0=ex0[:, :], in1=ex1[:, :])
                ex2 = scr()
                S.activation(out=ex2[:, :], in_=pc[2], func=AF.Exp)
                es = scr()
                V.tensor_add(out=es[:, :], in0=es01[:, :], in1=ex2[:, :])
                lse = scr()
                S.activation(out=lse[:, :], in_=es[:, :], func=AF.Ln)
                tgu = ps.tile([Hp, AW], U32, tag="tmpU", name="tgu", bufs=1)
                _ts_u32imm(V, tgu[:, :], pmu[:, :], TGT_MASK, OP.bitwise_and)
                tgtf = scr()
                V.tensor_copy(out=tgtf[:, :], in_=tgu[:, :])
                # pick = pcls[tgt] via exact one-hot blend
                eqc1 = scr("tmpB")
                V.tensor_single_scalar(out=eqc1[:, :], in_=tgtf[:, :], scalar=1.0, op=OP.is_equal)
                eqc2 = scr("tmpB")
                V.tensor_single_scalar(out=eqc2[:, :], in_=tgtf[:, :], scalar=2.0, op=OP.is_equal)
                eq12 = scr("tmpB")
                V.tensor_add(out=eq12[:, :], in0=eqc1[:, :], in1=eqc2[:, :])
                eqc0 = scr("tmpB")
                V.tensor_scalar(out=eqc0[:, :], in0=eq12[:, :], scalar1=-1.0,
                                scalar2=1.0, op0=OP.mult, op1=OP.add)
                s0 = scr("tmpB")
                V.tensor_mul(out=s0[:, :], in0=eqc0[:, :], in1=pc[0])
                s1 = scr("tmpB")
                V.tensor_mul(out=s1[:, :], in0=eqc1[:, :], in1=pc[1])
                s2 = scr("tmpB")
                V.tensor_mul(out=s2[:, :], in0=eqc2[:, :], in1=pc[2])
                s01 = scr("tmpB")
                V.tensor_add(out=s01[:, :], in0=s0[:, :], in1=s1[:, :])
                pick = scr()
                V.tensor_add(out=pick[:, :], in0=s01[:, :], in1=s2[:, :])
                clsper = scr()
                V.tensor_sub(out=clsper[:, :], in0=lse[:, :], in1=pick[:, :])
                cls_col = pt.tile([Hp, 1], F32, tag="cls_col", name="cls_col")
                junk = scr("junkN")
                V.tensor_mul(out=junk[:, :], in0=clsper[:, :], in1=pos_f[:, :])
                V.tensor_reduce(out=cls_col[:, :], in_=junk[:, :], axis=AX.X, op=OP.add)
                cls_t = tiny("cls_t")
                G.tensor_reduce(out=cls_t[:1, :1], in_=cls_col[:, :], axis=AX.C, op=OP.add)

                if STOP_STAGE < 6:
                    per_image_scalars.append(None)
                    continue
                # ---------- matched-payload select loop ----------
                idq = ps.tile([Hp, AW], U32, tag="tmpU", name="idq", bufs=1)
                _ts_u32imm(V, idq[:, :], pmu[:, :], IDX_MASK, OP.bitwise_and)
                idxf = pl.tile([Hp, AW], F32, tag="idxf", name="idxf")
                V.tensor_copy(out=idxf[:, :], in_=idq[:, :])
                xac = pl.tile([Hp, AW], F32, tag="xac", name="xac")
                yac = pl.tile([Hp, AW], F32, tag="yac", name="yac")
                wac = pl.tile([Hp, AW], F32, tag="wac", name="wac")
                hac = pl.tile([Hp, AW], F32, tag="hac", name="hac")
                V.memset(xac[:, :], 0.0)
                V.memset(yac[:, :], 0.0)
                G.memset(wac[:, :], 0.0)
                G.memset(hac[:, :], 0.0)
                for m in range(Mp):
                    eq = ps.tile([Hp, AW], F32, tag="tmpB", name="eqsel")
                    V.tensor_single_scalar(out=eq[:, :], in_=idxf[:, :],
                                           scalar=float((31 - m) << 2), op=OP.is_equal)
                    V.scalar_tensor_tensor(out=xac[:, :], in0=eq[:, :],
                                           scalar=gcx[:, m:m + 1], in1=xac[:, :],
                                           op0=OP.mult, op1=OP.add)
                    V.scalar_tensor_tensor(out=yac[:, :], in0=eq[:, :],
                                           scalar=gcy[:, m:m + 1], in1=yac[:, :],
                                           op0=OP.mult, op1=OP.add)
                    V.scalar_tensor_tensor(out=wac[:, :], in0=eq[:, :],
                                           scalar=lgw[:, m:m + 1], in1=wac[:, :],
                                           op0=OP.mult, op1=OP.add)
                    V.scalar_tensor_tensor(out=hac[:, :], in0=eq[:, :],
                                           scalar=lgh[:, m:m + 1], in1=hac[:, :],
                                           op0=OP.mult, op1=OP.add)

                # ---------- localization ----------
                px, py = pv[:, :, 0, :], pv[:, :, 1, :]
                pw, ph = pv[:, :, 2, :], pv[:, :, 3, :]
                lsum = None
                for dn, (acc, inv, off, pp_) in enumerate((
                        (xac, invaw, axw, px), (yac, invah, ayh, py),
                        (wac, logaw, None, pw), (hac, logah, None, ph))):
                    d = scr()
                    if off is not None:
                        gxw = scr("tmpB")
                        V.tensor_mul(out=gxw[:, :], in0=acc[:, :], in1=inv[:, :])
                        ux = scr("tmpB")
                        V.tensor_add(out=ux[:, :], in0=pp_, in1=off[:, :])
                        V.tensor_sub(out=d[:, :], in0=ux[:, :], in1=gxw[:, :])
                    else:
                        tw0 = scr("tmpB")
                        V.tensor_sub(out=tw0[:, :], in0=acc[:, :], in1=inv[:, :])
                        V.tensor_tensor(out=d[:, :], in0=pp_, in1=tw0[:, :], op=OP.subtract)
                    absd = scr("tmpB")
                    S.activation(out=absd[:, :], in_=d[:, :], func=AF.Abs)
                    mm = scr("tmpB")
                    V.tensor_scalar_min(out=mm[:, :], in0=absd[:, :], scalar1=1.0)
                    msq = scr("tmpB")
                    V.scalar_tensor_tensor(out=msq[:, :], in0=mm[:, :], scalar=0.5,
                                           in1=mm[:, :], op0=OP.mult, op1=OP.mult)
                    rr = scr("tmpB")
                    V.tensor_sub(out=rr[:, :], in0=absd[:, :], in1=mm[:, :])
                    cc = scr()
                    V.tensor_add(out=cc[:, :], in0=rr[:, :], in1=msq[:, :])
                    if lsum is None:
                        lsum = cc
                    else:
                        nsum = scr()
                        V.tensor_add(out=nsum[:, :], in0=lsum[:, :], in1=cc[:, :])
                        lsum = nsum
                loc_col = pt.tile([Hp, 1], F32, tag="loc_col", name="loc_col")
                junk = scr("junkN")
                V.tensor_mul(out=junk[:, :], in0=lsum[:, :], in1=pos_f[:, :])
                V.tensor_reduce(out=loc_col[:, :], in_=junk[:, :], axis=AX.X, op=OP.add)
                loc_t = tiny("loc_t")
                G.tensor_reduce(out=loc_t[:1, :1], in_=loc_col[:, :], axis=AX.C, op=OP.add)

                per_image_scalars.append((objpos_t, objneg_t, cls_t, loc_t, npos_t, kk))

            # ---------- combine images and write result ----------
            res_t = pt.tile([1, 8], F32, tag="res_t", name="res_t")
            V.memset(res_t[:, :], 0.0)
            per_image_scalars = [s for s in per_image_scalars if s is not None]
            if not per_image_scalars:
                nc.sync.dma_start(out=d_res[:, :], in_=res_t[:, :])

            def acc_into(slot, tiles):
                cur = tiles[0]
                for t in tiles[1:]:
                    nxt = pt.tile([1, 1], F32, tag=f"accf{slot}", name=f"accf{slot}")
                    V.tensor_add(out=nxt[:, :], in0=cur[:, :], in1=t[:, :])
                    cur = nxt
                V.tensor_copy(out=res_t[:1, slot:slot + 1], in_=cur[:, :])

            objs = []
            if per_image_scalars:
                for (opos, oneg, _, _, _, _) in per_image_scalars:
                    ot = pt.tile([1, 1], F32, tag="ot", name="ot")
                    V.tensor_add(out=ot[:, :], in0=opos[:, :], in1=oneg[:, :])
                    objs.append(ot)
                acc_into(0, objs)
                acc_into(1, [s[2] for s in per_image_scalars])
                acc_into(2, [s[3] for s in per_image_scalars])
                acc_into(3, [s[4] for s in per_image_scalars])
                nsels = []
                for (_, _, _, _, npt_, kk_) in per_image_scalars:
                    nt = pt.tile([1, 1], F32, tag="nt", name="nt")
                    V.tensor_add(out=nt[:, :], in0=npt_[:, :], in1=kk_[:, :])
                    nsels.append(nt)
                acc_into(4, nsels)
                nc.sync.dma_start(out=d_res[:, :], in_=res_t[:, :])

    nc.compile()
    return nc


def prep_inputs(pred, anchors, gt_boxes, gt_labels, n_img=N_IMG, Wp=W, Ap=A, Mp=M):
    """Host-side sharding + layout prep (pure reshape/broadcast/cast)."""
    Hp = H
    pred = np.ascontiguousarray(pred, dtype=np.float32)
    anchors = np.asarray(anchors, dtype=np.float32)
    gt_boxes = np.asarray(gt_boxes, dtype=np.float32)
    gt_labels = np.asarray(gt_labels)

    anc_pl = np.ascontiguousarray(
        anchors.reshape(Hp, Wp, Ap, 4).transpose(3, 0, 2, 1).reshape(4, Hp, Ap * Wp))
    ipat = np.ascontiguousarray(
        np.broadcast_to(((31 - np.arange(Mp, dtype=np.uint32)) << 2), (Hp, Mp)))

    in_maps = []
    n_cores = pred.shape[0] // n_img
    for c in range(n_cores):
        sl = slice(c * n_img, (c + 1) * n_img)
        gtb = gt_boxes[sl].transpose(0, 2, 1)          # [n_img, 4, M]
        gtb_r = np.ascontiguousarray(
            np.broadcast_to(gtb[:, :, None, :], (n_img, 4, Hp, Mp)).astype(np.float32))
        gtl_r = np.ascontiguousarray(
            np.broadcast_to(gt_labels[sl].astype(np.float32)[:, None, :], (n_img, Hp, Mp)))
        in_maps.append({
            "pred": np.ascontiguousarray(pred[sl]),
            "anc": anc_pl,
            "gtb": gtb_r,
            "gtl": gtl_r,
            "ipat": ipat,
        })
    return in_maps


def finalize(partials):
    """partials: list of [1,8] arrays per core -> final [4] f32 output."""
    tot = np.sum(np.stack([np.asarray(p).reshape(8) for p in partials]),
                 axis=0, dtype=np.float64)
    obj_s, cls_s, loc_s, total_pos, total_sel = tot[:5]
    obj_s, cls_s, loc_s = np.float32(obj_s), np.float32(cls_s), np.float32(loc_s)
    denom_pos = np.float32(max(total_pos, 1.0))
    denom_obj = np.float32(max(total_sel, 1.0))
    loss_loc = np.float32(loc_s / denom_pos)
    loss_cls = np.float32(cls_s / denom_pos)
    loss_obj = np.float32(obj_s / denom_obj)
    loss_total = np.float32(2.0 * loss_loc + 1.0 * loss_cls + 1.0 * loss_obj)
    return np.array([loss_obj, loss_cls, loss_loc, loss_total], dtype=np.float32)


_NC_CACHE = {}


def _get_nc():
    if "nc" not in _NC_CACHE:
        _NC_CACHE["nc"] = build_nc()
    return _NC_CACHE["nc"]


def run_with_results(pred, anchors, gt_boxes, gt_labels, trace=False, **kw):
    nc = _get_nc()
    in_maps = prep_inputs(pred, anchors, gt_boxes, gt_labels)
    res = run_bass_kernel_spmd(nc, in_maps, list(range(N_CORES)), trace=trace, **kw)
    out = finalize([res.results[c]["res"] for c in range(N_CORES)])
    return out, res


def kernel(pred, anchors, gt_boxes, gt_labels):
    return run_with_results(pred, anchors, gt_boxes, gt_labels)[0]

